# revision 1
# baseline (speedup 1.0000x reference)
import numpy as np
import ml_dtypes

import concourse.bass as bass
import concourse.bacc as bacc
import concourse.mybir as mybir
from concourse.tile import TileContext
from concourse.bass_utils import run_bass_kernel_spmd

B, DIM, L = 16, 1024, 4096
HEADS, DH = 16, 64
INNER = HEADS * DH
TOPK = 64
NCORES = 8
BPC = B // NCORES  # batches per core = 2
KC = DIM // 128    # 8 contraction chunks
NT = L // 512      # 8 N tiles
BF16 = mybir.dt.bfloat16
F32 = mybir.dt.float32

_CACHE = {}


def _build_mm_kernel():
    """Kernel A: per core, for 2 batches: kv = Wkv @ xc, q = Wq @ xq.

    Inputs (bf16): xc (2,1024,4096), xq (2,1024,4096), wkvt (1024,2048),
    wqt (1024,1024). Outputs fp32: kv (2,2048,4096), q (2,1024,4096).
    """
    nc = bacc.Bacc(None, target_bir_lowering=False)
    xc = nc.dram_tensor("xc", [BPC, DIM, L], BF16, kind="ExternalInput")
    xq = nc.dram_tensor("xq", [BPC, DIM, L], BF16, kind="ExternalInput")
    wkvt = nc.dram_tensor("wkvt", [DIM, 2 * INNER], BF16, kind="ExternalInput")
    wqt = nc.dram_tensor("wqt", [DIM, INNER], BF16, kind="ExternalInput")
    kv = nc.dram_tensor("kv", [BPC, 2 * INNER, L], F32, kind="ExternalOutput")
    q = nc.dram_tensor("q", [BPC, INNER, L], F32, kind="ExternalOutput")

    with TileContext(nc) as tc:
        with (
            tc.tile_pool(name="w", bufs=8) as wp,
            tc.tile_pool(name="x", bufs=8) as xp,
            tc.tile_pool(name="ps", bufs=8, space="PSUM") as pp,
            tc.tile_pool(name="st", bufs=4) as sp,
        ):
            # weights resident
            wkvt_sb = [wp.tile([128, 2 * INNER], BF16, tag="wkv", name=f"wkv{_}") for _ in range(KC)]
            wqt_sb = [wp.tile([128, INNER], BF16, tag="wq", name=f"wq{_}") for _ in range(KC)]
            for kc in range(KC):
                nc.sync.dma_start(out=wkvt_sb[kc], in_=wkvt[kc * 128:(kc + 1) * 128, :])
                nc.sync.dma_start(out=wqt_sb[kc], in_=wqt[kc * 128:(kc + 1) * 128, :])

            for b in range(BPC):
                for (x_dram, w_sb, out_dram, mtiles) in (
                    (xc, wkvt_sb, kv, 2 * INNER // 128),
                    (xq, wqt_sb, q, INNER // 128),
                ):
                    x_sb = [xp.tile([128, L], BF16, tag="xin", name=f"xin{_}") for _ in range(KC)]
                    for kc in range(KC):
                        nc.sync.dma_start(
                            out=x_sb[kc], in_=x_dram[b, kc * 128:(kc + 1) * 128, :])
                    for mt in range(mtiles):
                        for nt in range(NT):
                            ps = pp.tile([128, 512], F32, name="ps")
                            for kc in range(KC):
                                nc.tensor.matmul(
                                    out=ps,
                                    lhsT=w_sb[kc][:, mt * 128:(mt + 1) * 128],
                                    rhs=x_sb[kc][:, nt * 512:(nt + 1) * 512],
                                    start=(kc == 0), stop=(kc == KC - 1))
                            st = sp.tile([128, 512], F32, tag="stage", name="stage")
                            nc.scalar.copy(out=st, in_=ps)
                            nc.gpsimd.dma_start(
                                out=out_dram[b, mt * 128:(mt + 1) * 128,
                                             nt * 512:(nt + 1) * 512],
                                in_=st)
    nc.finalize()
    return nc


def _build_out_kernel(gamma: float):
    """Kernel B: final = gamma * (W_out @ ao) + qs_raw."""
    nc = bacc.Bacc(None, target_bir_lowering=False)
    ao = nc.dram_tensor("ao", [BPC, INNER, L], BF16, kind="ExternalInput")
    qs = nc.dram_tensor("qs", [BPC, DIM, L], F32, kind="ExternalInput")
    woutt = nc.dram_tensor("woutt", [INNER, DIM], BF16, kind="ExternalInput")
    fin = nc.dram_tensor("fin", [BPC, DIM, L], F32, kind="ExternalOutput")

    with TileContext(nc) as tc:
        with (
            tc.tile_pool(name="w", bufs=8) as wp,
            tc.tile_pool(name="x", bufs=8) as xp,
            tc.tile_pool(name="r", bufs=3) as rp,
            tc.tile_pool(name="ps", bufs=8, space="PSUM") as pp,
            tc.tile_pool(name="st", bufs=4) as sp,
        ):
            w_sb = [wp.tile([128, DIM], BF16, tag="w", name=f"w{_}") for _ in range(KC)]
            for kc in range(KC):
                nc.sync.dma_start(out=w_sb[kc], in_=woutt[kc * 128:(kc + 1) * 128, :])
            for b in range(BPC):
                x_sb = [xp.tile([128, L], BF16, tag="xin", name=f"xin{_}") for _ in range(KC)]
                for kc in range(KC):
                    nc.sync.dma_start(
                        out=x_sb[kc], in_=ao[b, kc * 128:(kc + 1) * 128, :])
                for mt in range(DIM // 128):
                    for nt in range(NT):
                        ps = pp.tile([128, 512], F32, name="ps")
                        for kc in range(KC):
                            nc.tensor.matmul(
                                out=ps,
                                lhsT=w_sb[kc][:, mt * 128:(mt + 1) * 128],
                                rhs=x_sb[kc][:, nt * 512:(nt + 1) * 512],
                                start=(kc == 0), stop=(kc == KC - 1))
                        res = rp.tile([128, 512], F32, tag="res", name="res")
                        nc.sync.dma_start(
                            out=res,
                            in_=qs[b, mt * 128:(mt + 1) * 128,
                                   nt * 512:(nt + 1) * 512])
                        st = sp.tile([128, 512], F32, tag="stage", name="stage")
                        nc.vector.scalar_tensor_tensor(
                            out=st, in0=ps, scalar=float(gamma),
                            op0=mybir.AluOpType.mult, in1=res,
                            op1=mybir.AluOpType.add)
                        nc.gpsimd.dma_start(
                            out=fin[b, mt * 128:(mt + 1) * 128,
                                    nt * 512:(nt + 1) * 512],
                            in_=st)
    nc.finalize()
    return nc


def _bf16(x):
    return np.asarray(x, np.float32).astype(ml_dtypes.bfloat16)


def _run(nc, in_maps):
    res = run_bass_kernel_spmd(nc, in_maps, list(range(NCORES)))
    return res.results


def kernel(context, query_source, gamma_c, beta_c, gamma_q, beta_q,
           W_kv, W_q, W_out, gamma):
    context = np.asarray(context, np.float32)
    query_source = np.asarray(query_source, np.float32)
    W_kv = np.asarray(W_kv, np.float32)
    W_q = np.asarray(W_q, np.float32)
    W_out = np.asarray(W_out, np.float32)
    g = float(np.asarray(gamma).reshape(-1)[0])

    def chan_norm(x, gam, bet):
        mean = x.mean(axis=1, keepdims=True, dtype=np.float32)
        var = x.var(axis=1, keepdims=True, dtype=np.float32)
        return (np.asarray(gam, np.float32) * (x - mean) /
                (np.sqrt(var) + 1e-6) + np.asarray(bet, np.float32))

    ctx_n = chan_norm(context, gamma_c, beta_c)
    qs_n = chan_norm(query_source, gamma_q, beta_q)

    if "mm" not in _CACHE:
        _CACHE["mm"] = _build_mm_kernel()
    nc_a = _CACHE["mm"]
    wkvt = _bf16(W_kv.T)
    wqt = _bf16(W_q.T)
    in_maps = []
    for c in range(NCORES):
        sl = slice(c * BPC, (c + 1) * BPC)
        in_maps.append({
            "xc": _bf16(ctx_n[sl]), "xq": _bf16(qs_n[sl]),
            "wkvt": wkvt, "wqt": wqt,
        })
    res_a = _run(nc_a, in_maps)
    kv = np.concatenate([r["kv"] for r in res_a], axis=0)  # (B, 2048, L)
    q = np.concatenate([r["q"] for r in res_a], axis=0)    # (B, 1024, L)

    # host: fold heads, l2norm, probe topk, gather, attention
    def fold(t):
        return t.reshape(B, HEADS, -1, L).reshape(B * HEADS, -1, L)

    k, v = np.split(kv, 2, axis=1)
    q = fold(q)
    k = fold(k)
    v = fold(v)

    def l2n(x):
        n = np.sqrt(np.sum(x * x, axis=1, keepdims=True))
        return x / np.maximum(n, 1e-12)

    q = l2n(q)
    k = l2n(k)
    qp = np.abs(q).sum(axis=2)                       # (BH, DH)
    score = np.einsum("bc,bcl->bl", qp, np.abs(k))   # (BH, L)
    top_idx = np.argpartition(score, L - TOPK, axis=1)[:, L - TOPK:]
    idx3 = np.broadcast_to(top_idx[:, None, :], (B * HEADS, DH, TOPK))
    k_sel = np.take_along_axis(k, idx3, axis=2)      # (BH, DH, TOPK)
    v_sel = np.take_along_axis(v, idx3, axis=2)

    qt = np.ascontiguousarray(q.transpose(0, 2, 1))  # (BH, L, DH)
    sim = np.matmul(qt, k_sel)                       # (BH, L, TOPK)
    sim -= sim.max(axis=-1, keepdims=True)
    e = np.exp(sim, dtype=np.float32)
    attn = e / e.sum(axis=-1, keepdims=True)
    out = np.matmul(attn, v_sel.transpose(0, 2, 1))  # (BH, L, DH)
    ao = out.reshape(B, HEADS, L, DH).transpose(0, 1, 3, 2).reshape(B, INNER, L)

    key_b = ("out", g)
    if key_b not in _CACHE:
        _CACHE[key_b] = _build_out_kernel(g)
    nc_b = _CACHE[key_b]
    woutt = _bf16(W_out.T)
    in_maps_b = []
    for c in range(NCORES):
        sl = slice(c * BPC, (c + 1) * BPC)
        in_maps_b.append({
            "ao": _bf16(ao[sl]),
            "qs": np.ascontiguousarray(query_source[sl]),
            "woutt": woutt,
        })
    res_b = _run(nc_b, in_maps_b)
    fin = np.concatenate([r["fin"] for r in res_b], axis=0)
    return fin.astype(np.float32)



# revision 2
# speedup vs baseline: 2.5939x; 2.5939x over previous
"""Fully-fused sparse-attention kernel for one NeuronCore (bpc batches).

Per batch, all on device:
  A: channel-norm(query_source) via ones-matmul column stats -> xn_q (bf16);
     q = Wq^T.T @ xn_q; per-(head,l) l2-normalize -> qn (spilled to DRAM);
     probe qp[d] = sum_l |qn[d,l]|
  B: channel-norm(context) -> xn_c (resident); k-mm pass1 streamed:
     score[h,l] = (sum_d qp[d]|k[d,l]|) * rsqrt(sum_{d in h} k[d,l]^2);
     exact top-64/head via 8x (max8 + max_index + match_replace)
  C: k-mm pass2 streamed: gather 64 cols/head (gpsimd indirect_copy),
     l2-normalize selected cols -> block-diag kblock (128,128) per head-pair
  D: v-mm streamed: gather cols, PE-transpose -> block-diag vblock (j,d)
  E: per (pair, 512-chunk): sim^T = kblock.T @ qn; e = exp(sim);
     denom = onesblk.T @ e; attn = e * bcast(1/denom); ao = vblock.T @ attn
  F: fin = gamma * (Wo^T.T @ ao) + query_source  (fp32)

SBUF quadrant rule: every compute-engine SBUF AP starts at partition 0/32/64/96.
Constants come from DRAM; packed per-chunk stats live on single-partition rows.
"""
import numpy as np
from contextlib import ExitStack

import concourse.bass as bass
import concourse.bacc as bacc
import concourse.mybir as mybir
from concourse.tile import TileContext

F32 = mybir.dt.float32
BF16 = mybir.dt.bfloat16
U16 = mybir.dt.uint16
AF = mybir.ActivationFunctionType
ALU = mybir.AluOpType
AX = mybir.AxisListType

DIM = 1024
HEADS = 16
DH = 64
KT = DIM // 128
MT = DIM // 128
NPAIR = HEADS // 2


def make_consts():
    """Host-side constant tensors DMA'd into SBUF.

    cstA (128, 134) bf16: [e2sum 0:2 | onesblk 2:4 | id128 4:132 | ones_k 132 | ones64 133]
    cstB (2, 256) bf16:   [e2blk 0:128 | ones-row 128:256]
    """
    import ml_dtypes
    bf = ml_dtypes.bfloat16
    A = np.zeros((128, 134), np.float32)
    A[0:64, 0] = 1.0       # e2sum col0
    A[64:128, 1] = 1.0     # e2sum col1
    A[0:64, 2] = 1.0       # onesblk col0
    A[64:128, 3] = 1.0     # onesblk col1
    A[:, 4:132] = np.eye(128)
    A[:, 132] = 1.0        # ones_k
    A[:, 133] = 1.0        # ones64 (rows 0:64)
    B = np.zeros((2, 256), np.float32)
    B[0, 0:64] = 1.0       # e2blk row0
    B[1, 64:128] = 1.0     # e2blk row1
    B[0, 128:256] = 1.0    # ones1 row
    return A.astype(bf), B.astype(bf)


def build(gamma: float, L: int = 4096, bpc: int = 2):
    NT = L // 512
    nc = bacc.Bacc(None, target_bir_lowering=False)
    xq = nc.dram_tensor("xq", [bpc, DIM, L], F32, kind="ExternalInput")
    xc = nc.dram_tensor("xc", [bpc, DIM, L], F32, kind="ExternalInput")
    wqt = nc.dram_tensor("wqt", [DIM, DIM], BF16, kind="ExternalInput")
    wkt = nc.dram_tensor("wkt", [DIM, DIM], BF16, kind="ExternalInput")
    wvt = nc.dram_tensor("wvt", [DIM, DIM], BF16, kind="ExternalInput")
    wot = nc.dram_tensor("wot", [DIM, DIM], BF16, kind="ExternalInput")
    cstA_d = nc.dram_tensor("cstA", [128, 134], BF16, kind="ExternalInput")
    cstB_d = nc.dram_tensor("cstB", [2, 256], BF16, kind="ExternalInput")
    fin = nc.dram_tensor("fin", [bpc, DIM, L], F32, kind="ExternalOutput")

    with TileContext(nc) as tc, ExitStack() as ctx, \
            nc.allow_low_precision(reason="bf16 norm factors within rel-err budget"):
        P = lambda name, bufs, **kw: ctx.enter_context(
            tc.tile_pool(name=name, bufs=bufs, **kw))
        const_p = P("const", 1)
        w_p = P("w", 1)
        xin_p = P("xin", 3)
        big_p = P("big", 8)        # xn tiles then ao tiles (128, L) bf16
        xs_p = P("xs", 2)          # ktile / vtile scratch (128, L) bf16
        t_p = P("tn", 2)           # norm intermediate (128, 512) f32
        qc_p = P("qc", 2)          # qn chunk staging (128, 512) bf16
        qe_p = P("qe", 3)          # qn reload chunks (128, 512) bf16
        bc_p = P("bcst", 1)        # invB/minvB (128, L) bf16
        sc_p = P("scr", 4)         # small evac scratch
        st_p = P("stat", 4)        # small stats tiles
        row_p = P("rows", 1)       # score (16, L) f32, idx, wrap
        sm_p = P("srowm", 2)       # per-m score staging (2, L) f32
        blk_p = P("blk", 9)        # kblock/vblock (128,128) bf16
        e_p = P("eat", 3)          # e / attn (128, 512) bf16
        fin_p = P("fin", 3)        # output staging (128, 512) f32
        dram_p = P("dram", 2, space="DRAM")
        ps_mm = P("pmm", 4, space="PSUM")
        ps_st = P("pst", 2, space="PSUM")
        ps_sm = P("psm", 2, space="PSUM")

        cstA = const_p.tile([128, 134], BF16, name="cstA")
        nc.sync.dma_start(out=cstA, in_=cstA_d[:, :])
        cstB = const_p.tile([2, 256], BF16, name="cstB")
        nc.sync.dma_start(out=cstB, in_=cstB_d[:, :])
        e2sum = cstA[:, 0:2]
        onesblk = cstA[:, 2:4]
        identity128 = cstA[:, 4:132]
        ones_k = cstA[:, 132:133]
        ones64 = cstA[0:64, 133:134]
        e2blk = cstB[:, 0:128]
        ones1 = cstB[0:1, 128:256]
        ones1x64 = cstB[0:1, 128:192]

        def load_w(dram):
            w = w_p.tile([128, KT, DIM], BF16, tag="w", name="w")
            for k in range(KT):
                nc.sync.dma_start(out=w[:, k, :], in_=dram[128 * k:128 * (k + 1), :])
            return w

        XCH = min(1024, L)

        def chan_norm(x_dram, b):
            """Stream (1024, L) fp32 -> normalized bf16 tiles (in big_p)."""
            xb_tiles = []
            for m in range(MT):
                xb = big_p.tile([128, L], BF16, tag="big", name="xb")
                for c in range(L // XCH):
                    xf = xin_p.tile([128, XCH], F32, tag="xin", name="xin")
                    nc.sync.dma_start(
                        out=xf, in_=x_dram[b, 128 * m:128 * (m + 1),
                                           XCH * c:XCH * (c + 1)])
                    nc.scalar.activation(out=xb[:, XCH * c:XCH * (c + 1)], in_=xf,
                                         func=AF.Copy)
                xb_tiles.append(xb)
            # packed single-partition stat row: srow[0, 512n..] = colsum(x) chunk n,
            # srow[0, L + 512n..] = colsum(x^2) chunk n
            srow = st_p.tile([1, 2 * L], F32, tag="srow", name="srow")
            for n in range(NT):
                accA = ps_st.tile([1, 512], F32, tag="pst", name="accA")
                accB = ps_st.tile([1, 512], F32, tag="pst", name="accB")
                for m in range(MT):
                    sl = slice(512 * n, 512 * (n + 1))
                    xsq = sc_p.tile([128, 512], BF16, tag="sq", name="xsqc")
                    nc.scalar.activation(out=xsq, in_=xb_tiles[m][:, sl],
                                         func=AF.Square)
                    nc.tensor.matmul(out=accA, lhsT=ones_k, rhs=xb_tiles[m][:, sl],
                                     start=(m == 0), stop=(m == MT - 1))
                    nc.tensor.matmul(out=accB, lhsT=ones_k, rhs=xsq,
                                     start=(m == 0), stop=(m == MT - 1))
                nc.vector.tensor_copy(out=srow[:, 512 * n:512 * (n + 1)], in_=accA)
                nc.vector.tensor_copy(out=srow[:, L + 512 * n:L + 512 * (n + 1)],
                                      in_=accB)
            # mean | E[x^2] ; then nrow[0:L] = mean*inv, nrow[L:2L] = inv (bf16)
            nc.vector.tensor_scalar(out=srow, in0=srow, scalar1=1.0 / DIM,
                                    scalar2=None, op0=ALU.mult)
            var = st_p.tile([1, L], F32, tag="var", name="var")
            msq = st_p.tile([1, L], F32, tag="msq", name="msq")
            nc.vector.scalar_tensor_tensor(out=msq, in0=srow[:, 0:L], scalar=1.0,
                                           op0=ALU.bypass, in1=srow[:, 0:L],
                                           op1=ALU.mult)
            nc.vector.scalar_tensor_tensor(out=var, in0=srow[:, L:2 * L], scalar=1.0,
                                           op0=ALU.bypass, in1=msq, op1=ALU.subtract)
            nc.scalar.activation(out=var, in_=var, func=AF.Sqrt)
            nc.vector.tensor_scalar(out=var, in0=var, scalar1=1e-6, scalar2=None,
                                    op0=ALU.add)
            nrow = st_p.tile([1, 2 * L], BF16, tag="nrow", name="nrow")
            nc.vector.reciprocal(out=nrow[:, L:2 * L], in_=var)
            nc.vector.scalar_tensor_tensor(out=nrow[:, 0:L], in0=srow[:, 0:L],
                                           scalar=1.0, op0=ALU.bypass,
                                           in1=nrow[:, L:2 * L], op1=ALU.mult)
            invB = bc_p.tile([128, L], BF16, tag="invB", name="invB")
            minvB = bc_p.tile([128, L], BF16, tag="minvB", name="minvB")
            for n in range(NT):
                sl = slice(512 * n, 512 * (n + 1))
                pb = ps_mm.tile([128, 512], F32, tag="mm", name="pb")
                nc.tensor.matmul(out=pb, lhsT=ones1,
                                 rhs=nrow[:, L + 512 * n:L + 512 * (n + 1)],
                                 start=True, stop=True)
                nc.vector.tensor_copy(out=invB[:, sl], in_=pb)
                pb2 = ps_mm.tile([128, 512], F32, tag="mm", name="pb2")
                nc.tensor.matmul(out=pb2, lhsT=ones1, rhs=nrow[:, sl],
                                 start=True, stop=True)
                nc.vector.tensor_copy(out=minvB[:, sl], in_=pb2)
            for m in range(MT):
                for n in range(NT):
                    sl = slice(512 * n, 512 * (n + 1))
                    t = t_p.tile([128, 512], F32, tag="tn", name="t")
                    nc.vector.scalar_tensor_tensor(out=t, in0=xb_tiles[m][:, sl],
                                                   scalar=1.0, op0=ALU.bypass,
                                                   in1=invB[:, sl], op1=ALU.mult)
                    nc.vector.scalar_tensor_tensor(out=xb_tiles[m][:, sl],
                                                   in0=minvB[:, sl], scalar=-1.0,
                                                   op0=ALU.mult, in1=t, op1=ALU.add)
            return xb_tiles

        for b in range(bpc):
            # ================= phase A: q side =================
            xnq = chan_norm(xq, b)
            wq = load_w(wqt)
            qn_dram = dram_p.tile([DIM, L], BF16, tag="qnd", name="qn_dram")
            qpE_tiles = []
            for m in range(MT):
                qp_parts = st_p.tile([128, NT], F32, tag="qpp", name="qp_parts")
                for n in range(NT):
                    qps = ps_mm.tile([128, 512], F32, tag="mm", name="qps")
                    for k in range(KT):
                        nc.tensor.matmul(out=qps, lhsT=wq[:, k, 128 * m:128 * (m + 1)],
                                         rhs=xnq[k][:, 512 * n:512 * (n + 1)],
                                         start=(k == 0), stop=(k == KT - 1))
                    qsq = sc_p.tile([128, 512], BF16, tag="sq", name="qsq")
                    nc.scalar.activation(out=qsq, in_=qps, func=AF.Square)
                    sps = ps_sm.tile([2, 512], F32, tag="sm", name="sps")
                    nc.tensor.matmul(out=sps, lhsT=e2sum, rhs=qsq, start=True,
                                     stop=True)
                    rqf = st_p.tile([2, 512], F32, tag="rqf", name="rqf")
                    nc.scalar.activation(out=rqf, in_=sps, func=AF.Sqrt)
                    rq = st_p.tile([2, 512], BF16, tag="rq", name="rq")
                    nc.vector.reciprocal(out=rq, in_=rqf)
                    rqB = ps_sm.tile([128, 512], F32, tag="sm", name="rqB")
                    nc.tensor.matmul(out=rqB, lhsT=e2blk, rhs=rq, start=True,
                                     stop=True)
                    qtmp = sc_p.tile([128, 512], BF16, tag="qt", name="qtmp")
                    nc.scalar.activation(out=qtmp, in_=qps, func=AF.Copy)
                    qnc = qc_p.tile([128, 512], BF16, tag="qc", name="qnc")
                    nc.vector.scalar_tensor_tensor(out=qnc, in0=qtmp, scalar=1.0,
                                                   op0=ALU.bypass, in1=rqB,
                                                   op1=ALU.mult)
                    nc.vector.tensor_reduce(out=qp_parts[:, n:n + 1], in_=qnc,
                                            axis=AX.X, op=ALU.add,
                                            apply_absolute_value=True)
                    nc.sync.dma_start(
                        out=qn_dram[128 * m:128 * (m + 1), 512 * n:512 * (n + 1)],
                        in_=qnc)
                qp = st_p.tile([128, 1], F32, tag="qp", name="qp")
                nc.vector.tensor_reduce(out=qp, in_=qp_parts, axis=AX.X, op=ALU.add)
                qpE = st_p.tile([128, 2], BF16, tag="qpE", name="qpE")
                nc.vector.memset(qpE, 0.0)
                nc.vector.tensor_copy(out=qpE[0:64, 0:1], in_=qp[0:64, :])
                nc.vector.tensor_copy(out=qpE[64:128, 1:2], in_=qp[64:128, :])
                qpE_tiles.append(qpE)

            # ========== phase B: ctx norm + k pass1 (score) + topk ==========
            xnc = chan_norm(xc, b)
            wk = load_w(wkt)
            score = row_p.tile([16, L], F32, tag="score", name="score")
            for m in range(MT):
                srm = sm_p.tile([2, L], F32, tag="srm", name="srm")
                for n in range(NT):
                    kps = ps_mm.tile([128, 512], F32, tag="mm", name="kps")
                    for k in range(KT):
                        nc.tensor.matmul(out=kps, lhsT=wk[:, k, 128 * m:128 * (m + 1)],
                                         rhs=xnc[k][:, 512 * n:512 * (n + 1)],
                                         start=(k == 0), stop=(k == KT - 1))
                    ksq = sc_p.tile([128, 512], BF16, tag="sq", name="ksq")
                    nc.scalar.activation(out=ksq, in_=kps, func=AF.Square)
                    kab = sc_p.tile([128, 512], BF16, tag="qt", name="kab")
                    nc.scalar.activation(out=kab, in_=kps, func=AF.Abs)
                    sps = ps_sm.tile([2, 512], F32, tag="sm", name="ksps")
                    nc.tensor.matmul(out=sps, lhsT=e2sum, rhs=ksq, start=True,
                                     stop=True)
                    rkf = st_p.tile([2, 512], F32, tag="rqf", name="rkf")
                    nc.scalar.activation(out=rkf, in_=sps, func=AF.Sqrt)
                    rk = st_p.tile([2, 512], BF16, tag="rq", name="rk")
                    nc.vector.reciprocal(out=rk, in_=rkf)
                    pps = ps_sm.tile([2, 512], F32, tag="sm", name="pps")
                    nc.tensor.matmul(out=pps, lhsT=qpE_tiles[m], rhs=kab,
                                     start=True, stop=True)
                    nc.vector.scalar_tensor_tensor(
                        out=srm[:, 512 * n:512 * (n + 1)],
                        in0=pps, scalar=1.0, op0=ALU.bypass, in1=rk, op1=ALU.mult)
                nc.sync.dma_start(out=score[2 * m:2 * m + 2, :], in_=srm)
            idx_sb = row_p.tile([16, 64], U16, tag="idx", name="idx")
            maxv = st_p.tile([16, 8], F32, tag="maxv", name="maxv")
            for r in range(8):
                nc.vector.max(out=maxv, in_=score)
                nc.vector.max_index(out=idx_sb[:, 8 * r:8 * r + 8], in_max=maxv,
                                    in_values=score)
                nc.vector.match_replace(out=score, in_to_replace=maxv,
                                        in_values=score, imm_value=-1e30)
            wrapall = row_p.tile([128, 64], U16, tag="wrap", name="wrap")
            for h in range(HEADS):
                for r in range(8):
                    nc.sync.dma_start(
                        out=wrapall[8 * (r % 2):8 * (r % 2) + 8,
                                    4 * h + r // 2:4 * h + r // 2 + 1],
                        in_=idx_sb[h:h + 1, 8 * r:8 * r + 8])
            for g in range(1, 8):
                nc.sync.dma_start(out=wrapall[16 * g:16 * g + 16, :],
                                  in_=wrapall[0:16, :])

            # ========== phase C: k pass2 -> kblocks ==========
            kblocks = []
            for m in range(MT):
                ktile = xs_p.tile([128, L], BF16, tag="xs", name="ktile")
                for n in range(NT):
                    kps = ps_mm.tile([128, 512], F32, tag="mm", name="kps2")
                    for k in range(KT):
                        nc.tensor.matmul(out=kps, lhsT=wk[:, k, 128 * m:128 * (m + 1)],
                                         rhs=xnc[k][:, 512 * n:512 * (n + 1)],
                                         start=(k == 0), stop=(k == KT - 1))
                    nc.scalar.activation(out=ktile[:, 512 * n:512 * (n + 1)], in_=kps,
                                         func=AF.Copy)
                kb = blk_p.tile([128, 128], BF16, tag="kb", name="kb")
                nc.vector.memset(kb, 0.0)
                for hh in range(2):
                    h = 2 * m + hh
                    gat = sc_p.tile([128, 64], BF16, tag="gat", name="gat")
                    nc.gpsimd.indirect_copy(out=gat, data=ktile,
                                            idxs=wrapall[:, 4 * h:4 * h + 4],
                                            i_know_ap_gather_is_preferred=True)
                    ksel = gat[64 * hh:64 * hh + 64, :]
                    ks2 = sc_p.tile([64, 64], BF16, tag="ks2", name="ks2")
                    nc.scalar.activation(out=ks2, in_=ksel, func=AF.Square)
                    ssp = ps_sm.tile([1, 64], F32, tag="sm", name="ssp")
                    nc.tensor.matmul(out=ssp, lhsT=ones64, rhs=ks2, start=True,
                                     stop=True)
                    rksf = st_p.tile([1, 64], F32, tag="rksf", name="rksf")
                    nc.scalar.activation(out=rksf, in_=ssp, func=AF.Sqrt)
                    rks = st_p.tile([1, 64], BF16, tag="rks", name="rks")
                    nc.vector.reciprocal(out=rks, in_=rksf)
                    rkB = ps_sm.tile([64, 64], F32, tag="sm", name="rkB")
                    nc.tensor.matmul(out=rkB, lhsT=ones1x64, rhs=rks, start=True,
                                     stop=True)
                    nc.vector.scalar_tensor_tensor(
                        out=kb[64 * hh:64 * hh + 64, 64 * hh:64 * hh + 64],
                        in0=ksel, scalar=1.0, op0=ALU.bypass, in1=rkB, op1=ALU.mult)
                kblocks.append(kb)

            # ========== phase D: v -> vblocks ==========
            wv = load_w(wvt)
            vblocks = []
            for m in range(MT):
                vtile = xs_p.tile([128, L], BF16, tag="xs", name="vtile")
                for n in range(NT):
                    vps = ps_mm.tile([128, 512], F32, tag="mm", name="vps")
                    for k in range(KT):
                        nc.tensor.matmul(out=vps, lhsT=wv[:, k, 128 * m:128 * (m + 1)],
                                         rhs=xnc[k][:, 512 * n:512 * (n + 1)],
                                         start=(k == 0), stop=(k == KT - 1))
                    nc.scalar.activation(out=vtile[:, 512 * n:512 * (n + 1)], in_=vps,
                                         func=AF.Copy)
                vb = blk_p.tile([128, 128], BF16, tag="vb", name="vb")
                nc.vector.memset(vb, 0.0)
                for hh in range(2):
                    h = 2 * m + hh
                    gat = sc_p.tile([128, 64], BF16, tag="gat", name="vgat")
                    nc.gpsimd.indirect_copy(out=gat, data=vtile,
                                            idxs=wrapall[:, 4 * h:4 * h + 4],
                                            i_know_ap_gather_is_preferred=True)
                    vt = ps_sm.tile([64, 128], BF16, tag="sm", name="vt")
                    nc.tensor.transpose(out=vt, in_=gat, identity=identity128)
                    nc.vector.tensor_copy(
                        out=vb[64 * hh:64 * hh + 64, 64 * hh:64 * hh + 64],
                        in_=vt[:, 64 * hh:64 * hh + 64])
                vblocks.append(vb)

            # ========== phase E: attention ==========
            ao_tiles = []
            for p in range(NPAIR):
                ao = big_p.tile([128, L], BF16, tag="big", name="ao")
                for n in range(NT):
                    qn = qe_p.tile([128, 512], BF16, tag="qe", name="qn")
                    nc.sync.dma_start(
                        out=qn, in_=qn_dram[128 * p:128 * (p + 1),
                                            512 * n:512 * (n + 1)])
                    sim = ps_mm.tile([128, 512], F32, tag="mm", name="sim")
                    nc.tensor.matmul(out=sim, lhsT=kblocks[p], rhs=qn,
                                     start=True, stop=True)
                    e = e_p.tile([128, 512], BF16, tag="e", name="e")
                    nc.scalar.activation(out=e, in_=sim, func=AF.Exp)
                    dps = ps_sm.tile([2, 512], F32, tag="sm", name="dps")
                    nc.tensor.matmul(out=dps, lhsT=onesblk, rhs=e, start=True,
                                     stop=True)
                    rd = st_p.tile([2, 512], BF16, tag="rq", name="rd")
                    nc.vector.reciprocal(out=rd, in_=dps)
                    rdB = ps_sm.tile([128, 512], F32, tag="sm", name="rdB")
                    nc.tensor.matmul(out=rdB, lhsT=e2blk, rhs=rd, start=True,
                                     stop=True)
                    attn = e_p.tile([128, 512], BF16, tag="at", name="attn")
                    nc.vector.scalar_tensor_tensor(out=attn, in0=e, scalar=1.0,
                                                   op0=ALU.bypass, in1=rdB,
                                                   op1=ALU.mult)
                    aop = ps_mm.tile([128, 512], F32, tag="mm", name="aop")
                    nc.tensor.matmul(out=aop, lhsT=vblocks[p], rhs=attn,
                                     start=True, stop=True)
                    nc.scalar.activation(out=ao[:, 512 * n:512 * (n + 1)], in_=aop,
                                         func=AF.Copy)
                ao_tiles.append(ao)

            # ========== phase F: Wout + residual ==========
            wo = load_w(wot)
            for m in range(MT):
                for n in range(NT):
                    ops_ = ps_mm.tile([128, 512], F32, tag="mm", name="ops")
                    for k in range(KT):
                        nc.tensor.matmul(out=ops_, lhsT=wo[:, k, 128 * m:128 * (m + 1)],
                                         rhs=ao_tiles[k][:, 512 * n:512 * (n + 1)],
                                         start=(k == 0), stop=(k == KT - 1))
                    qres = xin_p.tile([128, 512], F32, tag="xin", name="qres")
                    nc.sync.dma_start(
                        out=qres, in_=xq[b, 128 * m:128 * (m + 1),
                                         512 * n:512 * (n + 1)])
                    ftile = fin_p.tile([128, 512], F32, tag="fin", name="ftile")
                    nc.vector.scalar_tensor_tensor(out=ftile, in0=ops_,
                                                   scalar=float(gamma), op0=ALU.mult,
                                                   in1=qres, op1=ALU.add)
                    nc.gpsimd.dma_start(
                        out=fin[b, 128 * m:128 * (m + 1), 512 * n:512 * (n + 1)],
                        in_=ftile)

    nc.finalize()
    return nc




NCORES = 8
B = 16
L = 4096
BPC = B // NCORES

_CACHE = {}


def _bf16_t(w):
    import ml_dtypes
    return np.ascontiguousarray(np.asarray(w, np.float32).T).astype(ml_dtypes.bfloat16)


def kernel(context, query_source, gamma_c, beta_c, gamma_q, beta_q,
           W_kv, W_q, W_out, gamma):
    from concourse.bass_utils import run_bass_kernel_spmd

    context = np.asarray(context, np.float32)
    query_source = np.asarray(query_source, np.float32)
    W_kv = np.asarray(W_kv, np.float32)
    W_q = np.asarray(W_q, np.float32)
    W_out = np.asarray(W_out, np.float32)
    gc = np.asarray(gamma_c, np.float32).reshape(-1)
    gq = np.asarray(gamma_q, np.float32).reshape(-1)
    bc = np.asarray(beta_c, np.float32).reshape(-1)
    bq = np.asarray(beta_q, np.float32).reshape(-1)
    g = float(np.asarray(gamma).reshape(-1)[0])
    if np.abs(bc).max() > 1e-12 or np.abs(bq).max() > 1e-12:
        raise NotImplementedError("nonzero channel-norm beta not supported")

    # fold per-channel gamma into the projection weights (exact)
    wk_eff = W_kv[:DIM] * gc[None, :]
    wv_eff = W_kv[DIM:] * gc[None, :]
    wq_eff = W_q * gq[None, :]

    key = ("fused", g)
    if key not in _CACHE:
        _CACHE[key] = build(g, L=L, bpc=BPC)
    nc = _CACHE[key]

    cA, cB = make_consts()
    wqt = _bf16_t(wq_eff)
    wkt = _bf16_t(wk_eff)
    wvt = _bf16_t(wv_eff)
    wot = _bf16_t(W_out)
    in_maps = []
    for c in range(NCORES):
        sl = slice(c * BPC, (c + 1) * BPC)
        in_maps.append({
            "xq": query_source[sl], "xc": context[sl],
            "wqt": wqt, "wkt": wkt, "wvt": wvt, "wot": wot,
            "cstA": cA, "cstB": cB,
        })
    res = run_bass_kernel_spmd(nc, in_maps, list(range(NCORES)))
    fin = np.concatenate([r["fin"] for r in res.results], axis=0)
    return np.ascontiguousarray(fin, dtype=np.float32)
